# revision 1
# baseline (speedup 1.0000x reference)
"""GATv2 layer — data-parallel over batch B across 8 NeuronCores.

Full inputs in, full output out. x:[256,128,256] f32, adj:[128,128] i32,
W_l/W_r:[256,64], a:[64], W_out:[256,256]. Each core computes B/8=32
batches; adj and all weights are replicated.
"""
import numpy as np
import jax
import jax.numpy as jnp

B, V, C_IN, C_OUT, D = 256, 128, 256, 256, 64
M = 8


def _gat_shard(x, adj, W_l, W_r, a, W_out):
    # x: [B/M, V, C_IN]
    Wh = jnp.einsum('bvc,co->bvo', x, W_out)            # [b,V,C_out]
    e_l = jnp.einsum('bvc,cd->bvd', x, W_l)             # [b,V,D]
    e_r = jnp.einsum('bvc,cd->bvd', x, W_r)             # [b,V,D]
    # leaky_relu(z) = 0.2*z + 0.8*relu(z); the linear part separates, so
    # only the relu part needs the pairwise [b,V,V,D] intermediate.
    s_l = e_l @ a                                       # [b,V]
    s_r = e_r @ a                                       # [b,V]
    z = e_l[:, :, None, :] + e_r[:, None, :, :]         # [b,V,V,D]
    r = jnp.einsum('bijd,d->bij', jnp.maximum(z, 0.0), a)
    e = 0.2 * (s_l[:, :, None] + s_r[:, None, :]) + 0.8 * r
    e = jnp.where((adj == 0)[None, :, :], -jnp.inf, e)
    alpha = jax.nn.softmax(e, axis=2)                   # [b,V,V]
    out = jnp.einsum('bij,bjc->bic', alpha, Wh)         # [b,V,C_out]
    return jax.nn.elu(out)


_pm = jax.pmap(_gat_shard, in_axes=(0, None, None, None, None, None))


def kernel(x, adj, W_l, W_r, a, W_out):
    xs = np.asarray(x).reshape(M, B // M, V, C_IN)
    out = _pm(xs, jnp.asarray(adj), jnp.asarray(W_l), jnp.asarray(W_r),
              jnp.asarray(a), jnp.asarray(W_out))
    return np.asarray(out).reshape(B, V, C_OUT).astype(np.float32)

